# revision 14
# baseline (speedup 1.0000x reference)
"""Bahdanau-style attention scores kernel for Trainium2 (8 NeuronCores).

Reference computation (B=32, S=2048, ENC_H=512, DEC_H=1024):
    W_s = attn_w[:, :1024]; W_e = attn_w[:, 1024:]
    proj_s = s @ W_s.T                      # [B, 1024]
    proj_e = enc @ W_e.T                    # [B, S, 1024]
    scores = tanh(proj_s[:, None] + proj_e) @ v_w.T   # [B, S]
    out = softmax(scores, axis=1)

Strategy: data-parallel over batch (4 batches per core), no collectives.

Layout keeps the hidden dim h on PSUM partitions for the main matmul
    projT[h, s] = sum_e W_eT[e, h] * encT[e, s]
so the per-batch proj_s bias is a per-partition scalar fused into the ACT
tanh. proj_s itself is precomputed on the host (tiny), enc is pre-cast to
bf16 on the host, and the weight is pre-blocked into hc-major strips so
the first PE accumulation group is gated on one 0.7us DMA, not the whole
2 MB weight.

The v-dot (scores = v . tanh) uses 4-way PE column-group tiling: four
M=1 matmuls with tile_position (0, 32j) stream their tanh tiles through
four independent XBUSes concurrently, so 8 hc slices cost ~2 matmul
slots instead of 8. The four partial rows (PSUM partitions 0/32/64/96)
are collapsed by one extra M=1 matmul against a 4-hot indicator vector
after a DVE copy to SBUF. All cross-engine consumers are emitted one
loop step late (software pipelining) so the PE queue never waits on
ACT/DVE results.
"""

import numpy as np
import ml_dtypes

import concourse.bass as bass
import concourse.tile as tile
from concourse import mybir
from concourse.bass_utils import run_bass_kernel_spmd

N_CORES = 8
B, S = 32, 2048
E = 1024  # 2*ENC_H, contraction dim of the big matmul
H = 1024  # DEC_H, hidden dim of tanh
D = 1024  # DEC_H, contraction dim of proj_s
BPC = B // N_CORES  # batches per core
P = 128
EC, HC = E // P, H // P
SBLK = 512
NSB = S // SBLK

F32 = mybir.dt.float32
BF16 = mybir.dt.bfloat16
NP_BF16 = ml_dtypes.bfloat16

COLTILE = True  # 4-way column-group tiling for the v-dot

_cache = {}


def _split_multiwaits(nc):
    """Walrus in this toolchain rejects instructions carrying more than one
    semaphore wait ("Too many sync wait commands"). Engine queues dispatch in
    order, so moving the extra waits onto same-engine NoOps just before the
    instruction is semantically identical."""
    for fn in nc.m.functions:
        for blk in fn.blocks:
            out = []
            for inst in blk.instructions:
                si = inst.sync_info
                waits = list(si.on_wait) if si is not None and si.on_wait else []
                if len(waits) > 1:
                    for i, w in enumerate(waits[:-1]):
                        out.append(
                            mybir.InstNoOp(
                                name=f"{inst.name}-w{i}",
                                engine=inst.engine,
                                sync_info=mybir.SyncInfo(on_wait=[w], on_update=[]),
                                bass_nofuse=True,
                            )
                        )
                    si.on_wait = [waits[-1]]
                    inst.sync_info = si
                out.append(inst)
            try:
                blk.instructions = out
            except Exception:
                blk.set_instructions(out)


def _dedup_ldweights(nc):
    """Drop Ldweights that reload identical array state as the previous one
    (carrying their waits onto the next PE instruction)."""
    ndrop = 0
    for fn in nc.m.functions:
        for blk in fn.blocks:
            out = []
            loaded = None
            pending_waits = []
            for inst in blk.instructions:
                if getattr(inst, "engine", None) != mybir.EngineType.PE:
                    out.append(inst)
                    continue
                if pending_waits:
                    si = inst.sync_info or mybir.SyncInfo(on_wait=[], on_update=[])
                    si.on_wait = list(si.on_wait) + pending_waits
                    inst.sync_info = si
                    pending_waits = []
                if isinstance(inst, mybir.InstLdweights):
                    ap = inst.ins[0]
                    key = (
                        ap.memref,
                        ap.offset,
                        str(ap.ap),
                        str(ap.dtype),
                        str(getattr(inst, "tile_position", None)),
                    )
                    if key == loaded:
                        si = inst.sync_info
                        if si is not None and si.on_wait:
                            pending_waits = list(si.on_wait)
                        if si is not None and si.on_update:
                            out.append(inst)
                            continue
                        ndrop += 1
                        continue
                    loaded = key
                elif isinstance(inst, mybir.InstMatmult):
                    pass  # matmuls stream against loaded weights
                else:
                    loaded = None  # unknown PE instruction: be conservative
                out.append(inst)
            assert not pending_waits
            try:
                blk.instructions = out
            except Exception:
                blk.set_instructions(out)
    return ndrop


def _build_bass(post=True):
    nc = bass.Bass()
    enc_t = nc.dram_tensor("enc_t", [BPC, EC, P, S], BF16, kind="ExternalInput")
    w_t = nc.dram_tensor("w_t", [HC, P, EC, P], BF16, kind="ExternalInput")
    projs_t = nc.dram_tensor("projs_t", [P, HC, BPC], F32, kind="ExternalInput")
    v_t = nc.dram_tensor("v_t", [P, HC, 1], BF16, kind="ExternalInput")
    ind_t = nc.dram_tensor("ind_t", [P, 1], BF16, kind="ExternalInput")
    out = nc.dram_tensor("out", [BPC, S], F32, kind="ExternalOutput")

    Tanh = mybir.ActivationFunctionType.Tanh
    Exp = mybir.ActivationFunctionType.Exp

    with tile.TileContext(nc) as tc:
        with (
            tc.tile_pool(name="consts", bufs=1) as consts,
            tc.tile_pool(name="enc", bufs=3) as enc_pool,
            tc.tile_pool(name="tanh", bufs=12) as tanh_pool,
            tc.tile_pool(name="qcopy", bufs=2) as qcopy_pool,
            tc.tile_pool(name="rows", bufs=2) as row_pool,
            tc.tile_pool(name="mmps", bufs=4, space="PSUM") as mm_psum,
            tc.tile_pool(name="quad", bufs=2, space="PSUM") as quad_psum,
            tc.tile_pool(name="scps", bufs=2, space="PSUM") as sc_psum,
        ):
            # Small consts on the scalar HWDGE ring (instant), then that ring
            # helps with the first enc s-block. Weight strips go on the sync
            # ring; the bulk enc stream uses the gpsimd SWDGE path.
            projs_sb = consts.tile([P, HC, BPC], F32)
            nc.scalar.dma_start(out=projs_sb[:], in_=projs_t[:])
            v_sb = consts.tile([P, HC, 1], BF16)
            nc.scalar.dma_start(out=v_sb[:], in_=v_t[:])
            ind_sb = consts.tile([P, 1], BF16)
            nc.scalar.dma_start(out=ind_sb[:], in_=ind_t[:])

            # Weight strips: strip k holds W_eT[:, k*128:(k+1)*128] laid out
            # [p, ec, h'] so each partition line is one contiguous 2KB run.
            # Strip k is everything accumulation group hc=k needs, so the
            # first group is gated on one strip, and strip arrival (~0.73us)
            # outpaces group consumption (~1.73us).
            w_sb = consts.tile([P, EC, H], BF16)
            w_view = w_sb[:].rearrange("p ec (hc hp) -> p hc ec hp", hc=HC)
            for k in range(HC):
                nc.sync.dma_start(out=w_view[:, k, :, :], in_=w_t[k])

            quad_tile = None
            if COLTILE:
                # One persistent PSUM bank for the v-dot partials. The
                # collapse matmul reads all 128 partitions but the v-rounds
                # only ever write 4 of them; zero it once so stale PSUM
                # contents (potentially NaN) never reach the indicator dot
                # (0 * NaN = NaN). The WAR dependency (qcopy of s-block N
                # before v-round 0 of N+1) is ~8us wide, so reuse is free.
                quad_tile = quad_psum.tile([P, SBLK], F32, tag="quad")
                nc.vector.memset(quad_tile, 0.0)

            # Deferred closures, emitted one main-MM group later so the PE
            # queue never reaches an instruction whose producer (ACT tanh or
            # DVE copy) hasn't had a full group (~1.7us) to finish.
            stage_a = []  # v-round 1 (+ DVE quad copy)
            stage_b = []  # collapse matmul + exp
            finale = []  # per-batch softmax normalization

            def emit_one(lst):
                if lst:
                    lst.pop(0)()

            for b in range(BPC):
                encT = enc_pool.tile([P, EC, S], BF16)
                if b == 0:
                    # First s-block split across two idle DMA paths so all 8
                    # ec slices land within ~1.5us; remainder streams on the
                    # gpsimd path.
                    for ec in range(EC):
                        q = nc.scalar if ec % 2 == 0 else nc.gpsimd
                        q.dma_start(
                            out=encT[:, ec, 0:SBLK],
                            in_=enc_t[0, ec, :, 0:SBLK],
                        )
                    for ec in range(EC):
                        nc.gpsimd.dma_start(
                            out=encT[:, ec, SBLK:S],
                            in_=enc_t[0, ec, :, SBLK:S],
                        )
                else:
                    for ec in range(EC):
                        nc.gpsimd.dma_start(out=encT[:, ec, :], in_=enc_t[b, ec])

                exp_row = row_pool.tile([1, S], F32, tag="exp_row")
                sums = row_pool.tile([1, NSB], F32, tag="sums")

                for sb in range(NSB):
                    quad = quad_tile
                    sc_ps = sc_psum.tile([1, SBLK], F32, tag="scps")
                    ths = []

                    def v_round(ths4, first, quad=quad, sc_ps=sc_ps):
                        if COLTILE:
                            for j in range(4):
                                nc.tensor.matmul(
                                    quad[32 * j : 32 * j + 1, :],
                                    v_sb[:, (0 if first else 4) + j, :],
                                    ths4[j],
                                    start=first,
                                    stop=not first,
                                    tile_position=(0, 32 * j),
                                    skip_group_check=True,
                                )
                        else:
                            for j in range(4):
                                hc = (0 if first else 4) + j
                                nc.tensor.matmul(
                                    sc_ps,
                                    v_sb[:, hc, :],
                                    ths4[j],
                                    start=(first and j == 0),
                                    stop=(not first and j == 3),
                                    skip_group_check=True,
                                )

                    for hc in range(HC):
                        mm_ps = mm_psum.tile([P, SBLK], F32, tag="mmps")
                        for ec in range(EC):
                            nc.tensor.matmul(
                                mm_ps,
                                w_sb[:, ec, hc * P : (hc + 1) * P],
                                encT[:, ec, sb * SBLK : (sb + 1) * SBLK],
                                start=(ec == 0),
                                stop=(ec == EC - 1),
                            )
                        if hc == 1:
                            emit_one(stage_a)
                        if hc == 3:
                            emit_one(stage_b)
                        if hc == 5:
                            emit_one(finale)
                        th = tanh_pool.tile([P, SBLK], BF16, tag="tanh")
                        nc.scalar.activation(
                            th, mm_ps, Tanh, bias=projs_sb[:, hc, b : b + 1]
                        )
                        ths.append(th)
                        if hc == 6:
                            v_round(ths[0:4], True)

                    qbox = []

                    def part_a(ths=ths, quad=quad, v_round=v_round, qbox=qbox):
                        v_round(ths[4:8], False)
                        if COLTILE:
                            qcopy = qcopy_pool.tile([P, SBLK], BF16, tag="qcopy")
                            nc.vector.tensor_copy(qcopy, quad)
                            qbox.append(qcopy)

                    def part_b(
                        sc_ps=sc_ps,
                        qbox=qbox,
                        exp_row=exp_row,
                        sums=sums,
                        sb=sb,
                    ):
                        if COLTILE:
                            nc.tensor.matmul(
                                sc_ps, ind_sb[:], qbox[0], start=True, stop=True
                            )
                        nc.scalar.activation(
                            exp_row[:, sb * SBLK : (sb + 1) * SBLK],
                            sc_ps,
                            Exp,
                            accum_out=sums[:, sb : sb + 1],
                        )

                    stage_a.append(part_a)
                    stage_b.append(part_b)

                def batch_finale(b=b, exp_row=exp_row, sums=sums):
                    tot = row_pool.tile([1, 1], F32, tag="tot")
                    nc.vector.reduce_sum(tot, sums, axis=mybir.AxisListType.X)
                    rtot = row_pool.tile([1, 1], F32, tag="rtot")
                    nc.vector.reciprocal(rtot, tot)
                    out_row = row_pool.tile([1, S], F32, tag="out_row")
                    nc.vector.tensor_scalar_mul(out_row, exp_row, rtot)
                    nc.sync.dma_start(out=out[b : b + 1, :], in_=out_row[:])

                finale.append(batch_finale)

            while stage_a or stage_b or finale:
                emit_one(stage_a)
                emit_one(stage_b)
                emit_one(finale)

    if post:
        _dedup_ldweights(nc)
        _split_multiwaits(nc)
    return nc


def _prep_inputs(s, encoder_outputs, attn_w, v_w):
    s = np.asarray(s, dtype=np.float32)
    enc = np.asarray(encoder_outputs, dtype=np.float32)
    attn_w = np.asarray(attn_w, dtype=np.float32)
    v_w = np.asarray(v_w, dtype=np.float32)

    W_s = attn_w[:, :D]  # [H, D]
    W_e = attn_w[:, D:]  # [H, E]
    W_eT = np.ascontiguousarray(W_e.T)  # [E, H]
    # strip-blocked weight: [HC, P(e within tile), EC, P(h within strip)]
    w_t = np.ascontiguousarray(
        W_eT.reshape(EC, P, HC, P).transpose(2, 1, 0, 3)
    ).astype(NP_BF16)

    v_t = np.ascontiguousarray(v_w.reshape(HC, P).T).reshape(P, HC, 1).astype(NP_BF16)

    ind = np.zeros((P, 1), dtype=NP_BF16)
    ind[[0, 32, 64, 96], 0] = 1.0

    projs = s @ W_s.T  # [B, H] fp32 on host (tiny)

    in_maps = []
    for c in range(N_CORES):
        lo, hi = c * BPC, (c + 1) * BPC
        enc_c = np.ascontiguousarray(enc[lo:hi].transpose(0, 2, 1)).astype(NP_BF16)
        enc_c = enc_c.reshape(BPC, EC, P, S)
        projs_c = np.ascontiguousarray(
            projs[lo:hi].T.reshape(HC, P, BPC).transpose(1, 0, 2)
        ).astype(np.float32)
        in_maps.append(
            {
                "enc_t": enc_c,
                "w_t": w_t,
                "projs_t": projs_c,
                "v_t": v_t,
                "ind_t": ind,
            }
        )
    return in_maps


def _run(s, encoder_outputs, attn_w, v_w, trace=False):
    if "nc" not in _cache:
        _cache["nc"] = _build_bass()
    nc = _cache["nc"]
    in_maps = _prep_inputs(s, encoder_outputs, attn_w, v_w)
    res = run_bass_kernel_spmd(nc, in_maps, list(range(N_CORES)), trace=trace)
    out = np.concatenate([res.results[c]["out"] for c in range(N_CORES)], axis=0)
    return out.astype(np.float32), res


def kernel(s, encoder_outputs, attn_w, v_w):
    out, _ = _run(s, encoder_outputs, attn_w, v_w, trace=False)
    return out


# revision 20
# speedup vs baseline: 1.0834x; 1.0834x over previous
"""Bahdanau-style attention scores kernel for Trainium2 (8 NeuronCores).

Reference computation (B=32, S=2048, ENC_H=512, DEC_H=1024):
    W_s = attn_w[:, :1024]; W_e = attn_w[:, 1024:]
    proj_s = s @ W_s.T                      # [B, 1024]
    proj_e = enc @ W_e.T                    # [B, S, 1024]
    scores = tanh(proj_s[:, None] + proj_e) @ v_w.T   # [B, S]
    out = softmax(scores, axis=1)

Strategy: data-parallel over batch (4 batches per core), no collectives.

Layout keeps the hidden dim h on PSUM partitions for the main matmul
    projT[h, s] = sum_e W_eT[e, h] * encT[e, s]
so the per-batch proj_s bias is a per-partition scalar fused into the ACT
tanh. proj_s is precomputed on the host (tiny), enc is pre-cast to bf16
on the host, the weight arrives as 16 half-row descriptors with 1KB
contiguous runs, and the first two PSUM accumulation groups run
ec-outer so the PE starts ~1us after the DMA rings open instead of
waiting for the whole weight.

The v-dot (scores = v . tanh): all 32 M=1 matmuls of one batch form one
contiguous col-group-tiled region (tile_position (0,32j)), so the four
XBUS streams pipeline at the full-array rate and the full<->col-group
drain penalty (~0.6us) is paid once per batch instead of per matmul
group. The four partial rows of each s-block (PSUM partitions
0/32/64/96) are collapsed by a zero-padded full-array matmul against a
4-hot indicator matrix (full-array so it slots into the main stream
with no transition cost). All cross-engine consumers are emitted one
main-MM group late (software pipelining) so the PE queue never waits on
ACT/DVE results.
"""

import numpy as np
import ml_dtypes

import concourse.bass as bass
import concourse.tile as tile
from concourse import mybir
from concourse.bass_utils import run_bass_kernel_spmd

N_CORES = 8
B, S = 32, 2048
E = 1024  # 2*ENC_H, contraction dim of the big matmul
H = 1024  # DEC_H, hidden dim of tanh
D = 1024  # DEC_H, contraction dim of proj_s
BPC = B // N_CORES  # batches per core
P = 128
EC, HC = E // P, H // P
SBLK = 512
NSB = S // SBLK
HH = H // 2  # weight DMA half-row

F32 = mybir.dt.float32
BF16 = mybir.dt.bfloat16
NP_BF16 = ml_dtypes.bfloat16

_cache = {}


def _split_multiwaits(nc):
    """Walrus in this toolchain rejects instructions carrying more than one
    semaphore wait ("Too many sync wait commands"). Engine queues dispatch in
    order, so moving the extra waits onto same-engine NoOps just before the
    instruction is semantically identical."""
    for fn in nc.m.functions:
        for blk in fn.blocks:
            out = []
            for inst in blk.instructions:
                si = inst.sync_info
                waits = list(si.on_wait) if si is not None and si.on_wait else []
                if len(waits) > 1:
                    for i, w in enumerate(waits[:-1]):
                        out.append(
                            mybir.InstNoOp(
                                name=f"{inst.name}-w{i}",
                                engine=inst.engine,
                                sync_info=mybir.SyncInfo(on_wait=[w], on_update=[]),
                                bass_nofuse=True,
                            )
                        )
                    si.on_wait = [waits[-1]]
                    inst.sync_info = si
                out.append(inst)
            try:
                blk.instructions = out
            except Exception:
                blk.set_instructions(out)


def _dedup_ldweights(nc):
    """Drop Ldweights that reload identical array state as the previous one
    (carrying their waits onto the next PE instruction)."""
    ndrop = 0
    for fn in nc.m.functions:
        for blk in fn.blocks:
            out = []
            loaded = None
            pending_waits = []
            for inst in blk.instructions:
                if getattr(inst, "engine", None) != mybir.EngineType.PE:
                    out.append(inst)
                    continue
                if pending_waits:
                    si = inst.sync_info or mybir.SyncInfo(on_wait=[], on_update=[])
                    si.on_wait = list(si.on_wait) + pending_waits
                    inst.sync_info = si
                    pending_waits = []
                if isinstance(inst, mybir.InstLdweights):
                    ap = inst.ins[0]
                    key = (
                        ap.memref,
                        ap.offset,
                        str(ap.ap),
                        str(ap.dtype),
                        str(getattr(inst, "tile_position", None)),
                    )
                    if key == loaded:
                        si = inst.sync_info
                        if si is not None and si.on_wait:
                            pending_waits = list(si.on_wait)
                        if si is not None and si.on_update:
                            out.append(inst)
                            continue
                        ndrop += 1
                        continue
                    loaded = key
                elif isinstance(inst, mybir.InstMatmult):
                    pass  # matmuls stream against loaded weights
                else:
                    loaded = None  # unknown PE instruction: be conservative
                out.append(inst)
            assert not pending_waits
            try:
                blk.instructions = out
            except Exception:
                blk.set_instructions(out)
    return ndrop


def _build_bass(post=True):
    nc = bass.Bass()
    enc_t = nc.dram_tensor("enc_t", [BPC, EC, P, S], BF16, kind="ExternalInput")
    # weight half-rows: [half, ec, p(e), 512(h)] so every descriptor writes
    # 1KB contiguous per partition
    w_t = nc.dram_tensor("w_t", [2, EC, P, HH], BF16, kind="ExternalInput")
    projs_t = nc.dram_tensor("projs_t", [P, HC, BPC], F32, kind="ExternalInput")
    v_t = nc.dram_tensor("v_t", [P, HC, 1], BF16, kind="ExternalInput")
    ind_t = nc.dram_tensor("ind_t", [P, P], BF16, kind="ExternalInput")
    out = nc.dram_tensor("out", [BPC, S], F32, kind="ExternalOutput")

    Tanh = mybir.ActivationFunctionType.Tanh
    Exp = mybir.ActivationFunctionType.Exp

    with tile.TileContext(nc) as tc:
        with (
            tc.tile_pool(name="consts", bufs=1) as consts,
            tc.tile_pool(name="enc", bufs=2) as enc_pool,
            tc.tile_pool(name="tanh", bufs=38) as tanh_pool,
            tc.tile_pool(name="qcopy", bufs=8) as qcopy_pool,
            tc.tile_pool(name="rows", bufs=2) as row_pool,
            tc.tile_pool(name="mmps", bufs=2, space="PSUM") as mm_psum,
            tc.tile_pool(name="quad", bufs=4, space="PSUM") as quad_psum,
            tc.tile_pool(name="cps", bufs=2, space="PSUM") as c_psum,
        ):
            # DMA queue plan (first use decides the critical path):
            #   sync ring:   16 weight half-row descriptors
            #   gpsimd SWDGE: even-ec first s-block slices, then the rest of
            #                the enc stream
            #   scalar ring: projs (needed by the first tanh), odd-ec first
            #                s-block slices, v, indicator
            w_sb = consts.tile([P, EC, H], BF16)
            for half in range(2):
                for ec in range(EC):
                    nc.sync.dma_start(
                        out=w_sb[:, ec, half * HH : (half + 1) * HH],
                        in_=w_t[half, ec],
                    )

            projs_sb = consts.tile([P, HC, BPC], F32)
            nc.scalar.dma_start(out=projs_sb[:], in_=projs_t[:])

            quads = []
            for _ in range(NSB):
                q = quad_psum.tile([P, SBLK], F32, tag="quad")
                # The collapse matmul reads all 128 partitions but the
                # v-rounds only ever write 4; zero once so stale PSUM
                # contents never reach the indicator dot.
                nc.vector.memset(q, 0.0)
                quads.append(q)

            # Deferred closures, emitted one main-MM group later so the PE
            # queue never reaches an instruction whose producer (ACT tanh or
            # DVE copy) hasn't had a full group (~1.7us) to finish.
            slots = {}

            def emit_slot(key):
                fns = slots.pop(key, None)
                if fns:
                    for fn in fns:
                        fn()

            def defer(key, fn):
                slots.setdefault(key, []).append(fn)

            v_sb = None
            ind_sb = None

            def dma_enc(b):
                encT = enc_pool.tile([P, EC, S], BF16)
                if b == 0:
                    for ec in range(EC):
                        q = nc.gpsimd if ec % 2 == 0 else nc.scalar
                        q.dma_start(
                            out=encT[:, ec, 0:SBLK], in_=enc_t[0, ec, :, 0:SBLK]
                        )
                    for ec in range(EC):
                        nc.gpsimd.dma_start(
                            out=encT[:, ec, SBLK:S], in_=enc_t[0, ec, :, SBLK:S]
                        )
                else:
                    for ec in range(EC):
                        nc.gpsimd.dma_start(out=encT[:, ec, :], in_=enc_t[b, ec])
                return encT

            for b in range(BPC):
                encT = dma_enc(b)
                if b == 0:
                    # v / indicator consts ride the scalar ring after the
                    # first s-block's enc slices (not needed until ~15us).
                    v_sb = consts.tile([P, HC, 1], BF16)
                    nc.scalar.dma_start(out=v_sb[:], in_=v_t[:])
                    ind_sb = consts.tile([P, P], BF16)
                    nc.scalar.dma_start(out=ind_sb[:], in_=ind_t[:])

                exp_row = row_pool.tile([1, S], F32, tag="exp_row")
                sums = row_pool.tile([1, NSB], F32, tag="sums")
                ths_b = []

                gi = 0  # main-MM group counter within this batch
                for sb in range(NSB):
                    ths = []
                    if b == 0 and sb == 0:
                        # ec-outer phase for the first two groups: banks from
                        # the (idle) collapse pool accumulate as the weight /
                        # enc descriptors land, so the PE starts on the first
                        # descriptor instead of the eighth.
                        ph = [
                            c_psum.tile([P, SBLK], F32, tag="cps", name=f"ph{k}")
                            for k in range(2)
                        ]
                        for ec in range(EC):
                            for hc in range(2):
                                nc.tensor.matmul(
                                    ph[hc],
                                    w_sb[:, ec, hc * P : (hc + 1) * P],
                                    encT[:, ec, 0:SBLK],
                                    start=(ec == 0),
                                    stop=(ec == EC - 1),
                                    skip_group_check=True,
                                )
                        for hc in range(2):
                            th = tanh_pool.tile([P, SBLK], BF16, tag="tanh")
                            nc.scalar.activation(
                                th, ph[hc], Tanh, bias=projs_sb[:, hc, 0:1]
                            )
                            ths.append(th)
                        hc_range = range(2, HC)
                    else:
                        hc_range = range(HC)
                    for hc in hc_range:
                        mm_ps = mm_psum.tile([P, SBLK], F32, tag="mmps")
                        for ec in range(EC):
                            nc.tensor.matmul(
                                mm_ps,
                                w_sb[:, ec, hc * P : (hc + 1) * P],
                                encT[:, ec, sb * SBLK : (sb + 1) * SBLK],
                                start=(ec == 0),
                                stop=(ec == EC - 1),
                            )
                        emit_slot((b, gi))
                        gi += 1
                        th = tanh_pool.tile([P, SBLK], BF16, tag="tanh")
                        nc.scalar.activation(
                            th, mm_ps, Tanh, bias=projs_sb[:, hc, b : b + 1]
                        )
                        ths.append(th)
                    ths_b.append(ths)

                def v_region(ths_b=ths_b, b=b):
                    for r in range(2):
                        for sb in range(NSB):
                            for j in range(4):
                                hc = 4 * r + j
                                nc.tensor.matmul(
                                    quads[sb][32 * j : 32 * j + 1, :],
                                    v_sb[:, hc, :],
                                    ths_b[sb][hc],
                                    start=(r == 0),
                                    stop=(r == 1),
                                    tile_position=(0, 32 * j),
                                    skip_group_check=True,
                                )

                def qcopy_sb(sb):
                    qc = qcopy_pool.tile([P, SBLK], BF16, tag="qcopy")
                    nc.vector.tensor_copy(qc, quads[sb])
                    return qc

                def collapse_sb(sb, qc, exp_row=exp_row, sums=sums):
                    sc = c_psum.tile([P, SBLK], F32, tag="cps")
                    nc.tensor.matmul(sc, ind_sb[:], qc, start=True, stop=True)
                    nc.scalar.activation(
                        exp_row[:, sb * SBLK : (sb + 1) * SBLK],
                        sc[0:1, :],
                        Exp,
                        accum_out=sums[:, sb : sb + 1],
                    )

                def batch_finale(b=b, exp_row=exp_row, sums=sums):
                    tot = row_pool.tile([1, 1], F32, tag="tot")
                    nc.vector.reduce_sum(tot, sums, axis=mybir.AxisListType.X)
                    rtot = row_pool.tile([1, 1], F32, tag="rtot")
                    nc.vector.reciprocal(rtot, tot)
                    out_row = row_pool.tile([1, S], F32, tag="out_row")
                    nc.vector.tensor_scalar_mul(out_row, exp_row, rtot)
                    nc.sync.dma_start(out=out[b : b + 1, :], in_=out_row[:])

                # schedule this batch's postlude into the next batch's
                # main-MM stream (group index g of batch b+1)
                nb = b + 1

                def sched(g, fn):
                    if nb < BPC:
                        defer((nb, g), fn)
                    else:
                        defer(("tail", 0), fn)

                qbox = {}
                sched(1, v_region)
                for i, sb in enumerate(range(NSB)):
                    sched(2 + i, (lambda sb=sb: qbox.__setitem__(sb, qcopy_sb(sb))))
                for i, sb in enumerate(range(NSB)):
                    sched(3 + i, (lambda sb=sb: collapse_sb(sb, qbox[sb])))
                sched(7, batch_finale)

            emit_slot(("tail", 0))

    if post:
        _dedup_ldweights(nc)
        _split_multiwaits(nc)
    return nc


def _prep_inputs(s, encoder_outputs, attn_w, v_w):
    s = np.asarray(s, dtype=np.float32)
    enc = np.asarray(encoder_outputs, dtype=np.float32)
    attn_w = np.asarray(attn_w, dtype=np.float32)
    v_w = np.asarray(v_w, dtype=np.float32)

    W_s = attn_w[:, :D]  # [H, D]
    W_e = attn_w[:, D:]  # [H, E]
    W_eT = np.ascontiguousarray(W_e.T)  # [E, H]
    # [2 halves, EC, P, 512]: descriptor (half, ec) is [128, 512] with 1KB
    # contiguous runs on both sides
    w_t = np.ascontiguousarray(
        W_eT.reshape(EC, P, 2, HH).transpose(2, 0, 1, 3)
    ).astype(NP_BF16)

    v_t = np.ascontiguousarray(v_w.reshape(HC, P).T).reshape(P, HC, 1).astype(NP_BF16)

    # indicator matrix: column 0 selects partitions {0,32,64,96}, all other
    # columns zero -> full-array collapse matmul writes the score into PSUM
    # row 0 and zeros elsewhere
    ind = np.zeros((P, P), dtype=NP_BF16)
    ind[[0, 32, 64, 96], 0] = 1.0

    projs = s @ W_s.T  # [B, H] fp32 on host (tiny)

    in_maps = []
    for c in range(N_CORES):
        lo, hi = c * BPC, (c + 1) * BPC
        enc_c = np.ascontiguousarray(enc[lo:hi].transpose(0, 2, 1)).astype(NP_BF16)
        enc_c = enc_c.reshape(BPC, EC, P, S)
        projs_c = np.ascontiguousarray(
            projs[lo:hi].T.reshape(HC, P, BPC).transpose(1, 0, 2)
        ).astype(np.float32)
        in_maps.append(
            {
                "enc_t": enc_c,
                "w_t": w_t,
                "projs_t": projs_c,
                "v_t": v_t,
                "ind_t": ind,
            }
        )
    return in_maps


def _run(s, encoder_outputs, attn_w, v_w, trace=False):
    if "nc" not in _cache:
        _cache["nc"] = _build_bass()
    nc = _cache["nc"]
    in_maps = _prep_inputs(s, encoder_outputs, attn_w, v_w)
    res = run_bass_kernel_spmd(nc, in_maps, list(range(N_CORES)), trace=trace)
    out = np.concatenate([res.results[c]["out"] for c in range(N_CORES)], axis=0)
    return out.astype(np.float32), res


def kernel(s, encoder_outputs, attn_w, v_w):
    out, _ = _run(s, encoder_outputs, attn_w, v_w, trace=False)
    return out


# revision 22
# speedup vs baseline: 1.1120x; 1.0264x over previous
"""Bahdanau-style attention scores kernel for Trainium2 (8 NeuronCores).

Reference computation (B=32, S=2048, ENC_H=512, DEC_H=1024):
    W_s = attn_w[:, :1024]; W_e = attn_w[:, 1024:]
    proj_s = s @ W_s.T                      # [B, 1024]
    proj_e = enc @ W_e.T                    # [B, S, 1024]
    scores = tanh(proj_s[:, None] + proj_e) @ v_w.T   # [B, S]
    out = softmax(scores, axis=1)

Strategy: data-parallel over batch (4 batches per core), no collectives.

Layout keeps the hidden dim h on PSUM partitions for the main matmul
    projT[h, s] = sum_e W_eT[e, h] * encT[e, s]
so the per-batch proj_s bias is a per-partition scalar fused into the ACT
tanh. proj_s is precomputed on the host (tiny), enc is pre-cast to bf16
on the host, the weight arrives as 16 half-row descriptors with 1KB
contiguous runs, and the first two PSUM accumulation groups run
ec-outer so the PE starts ~1us after the DMA rings open instead of
waiting for the whole weight.

The v-dot (scores = v . tanh): all 32 M=1 matmuls of one batch form one
contiguous col-group-tiled region (tile_position (0,32j)), so the four
XBUS streams pipeline at the full-array rate and the full<->col-group
drain penalty (~0.6us) is paid once per batch instead of per matmul
group. The four partial rows of each s-block (PSUM partitions
0/32/64/96) are collapsed by a zero-padded full-array matmul against a
4-hot indicator matrix (full-array so it slots into the main stream
with no transition cost). All cross-engine consumers are emitted one
main-MM group late (software pipelining) so the PE queue never waits on
ACT/DVE results.
"""

import numpy as np
import ml_dtypes

import concourse.bass as bass
import concourse.tile as tile
from concourse import mybir
from concourse.bass_utils import run_bass_kernel_spmd

N_CORES = 8
B, S = 32, 2048
E = 1024  # 2*ENC_H, contraction dim of the big matmul
H = 1024  # DEC_H, hidden dim of tanh
D = 1024  # DEC_H, contraction dim of proj_s
BPC = B // N_CORES  # batches per core
P = 128
EC, HC = E // P, H // P
SBLK = 512
NSB = S // SBLK
HH = H // 2  # weight DMA half-row

F32 = mybir.dt.float32
BF16 = mybir.dt.bfloat16
NP_BF16 = ml_dtypes.bfloat16

_cache = {}


def _split_multiwaits(nc):
    """Walrus in this toolchain rejects instructions carrying more than one
    semaphore wait ("Too many sync wait commands"). Engine queues dispatch in
    order, so moving the extra waits onto same-engine NoOps just before the
    instruction is semantically identical."""
    for fn in nc.m.functions:
        for blk in fn.blocks:
            out = []
            for inst in blk.instructions:
                si = inst.sync_info
                waits = list(si.on_wait) if si is not None and si.on_wait else []
                if len(waits) > 1:
                    for i, w in enumerate(waits[:-1]):
                        out.append(
                            mybir.InstNoOp(
                                name=f"{inst.name}-w{i}",
                                engine=inst.engine,
                                sync_info=mybir.SyncInfo(on_wait=[w], on_update=[]),
                                bass_nofuse=True,
                            )
                        )
                    si.on_wait = [waits[-1]]
                    inst.sync_info = si
                out.append(inst)
            try:
                blk.instructions = out
            except Exception:
                blk.set_instructions(out)


def _dedup_ldweights(nc):
    """Drop Ldweights that reload identical array state as the previous one
    (carrying their waits onto the next PE instruction)."""
    ndrop = 0
    for fn in nc.m.functions:
        for blk in fn.blocks:
            out = []
            loaded = None
            pending_waits = []
            for inst in blk.instructions:
                if getattr(inst, "engine", None) != mybir.EngineType.PE:
                    out.append(inst)
                    continue
                if pending_waits:
                    si = inst.sync_info or mybir.SyncInfo(on_wait=[], on_update=[])
                    si.on_wait = list(si.on_wait) + pending_waits
                    inst.sync_info = si
                    pending_waits = []
                if isinstance(inst, mybir.InstLdweights):
                    ap = inst.ins[0]
                    key = (
                        ap.memref,
                        ap.offset,
                        str(ap.ap),
                        str(ap.dtype),
                        str(getattr(inst, "tile_position", None)),
                    )
                    if key == loaded:
                        si = inst.sync_info
                        if si is not None and si.on_wait:
                            pending_waits = list(si.on_wait)
                        if si is not None and si.on_update:
                            out.append(inst)
                            continue
                        ndrop += 1
                        continue
                    loaded = key
                elif isinstance(inst, mybir.InstMatmult):
                    pass  # matmuls stream against loaded weights
                else:
                    loaded = None  # unknown PE instruction: be conservative
                out.append(inst)
            assert not pending_waits
            try:
                blk.instructions = out
            except Exception:
                blk.set_instructions(out)
    return ndrop


def _build_bass(post=True):
    nc = bass.Bass()
    enc_t = nc.dram_tensor("enc_t", [BPC, EC, P, S], BF16, kind="ExternalInput")
    # weight half-rows: [half, ec, p(e), 512(h)] so every descriptor writes
    # 1KB contiguous per partition
    w_t = nc.dram_tensor("w_t", [2, EC, P, HH], BF16, kind="ExternalInput")
    projs_t = nc.dram_tensor("projs_t", [P, HC, BPC], F32, kind="ExternalInput")
    v_t = nc.dram_tensor("v_t", [P, HC, 1], BF16, kind="ExternalInput")
    ind_t = nc.dram_tensor("ind_t", [P, P], BF16, kind="ExternalInput")
    out = nc.dram_tensor("out", [BPC, S], F32, kind="ExternalOutput")

    Tanh = mybir.ActivationFunctionType.Tanh
    Exp = mybir.ActivationFunctionType.Exp

    with tile.TileContext(nc) as tc:
        with (
            tc.tile_pool(name="consts", bufs=1) as consts,
            tc.tile_pool(name="enc", bufs=2) as enc_pool,
            tc.tile_pool(name="tanh", bufs=38) as tanh_pool,
            tc.tile_pool(name="qcopy", bufs=8) as qcopy_pool,
            tc.tile_pool(name="rows", bufs=2) as row_pool,
            tc.tile_pool(name="mmps", bufs=2, space="PSUM") as mm_psum,
            tc.tile_pool(name="quad", bufs=4, space="PSUM") as quad_psum,
            tc.tile_pool(name="cps", bufs=2, space="PSUM") as c_psum,
        ):
            # DMA queue plan (first use decides the critical path):
            #   sync ring:   16 weight half-row descriptors
            #   gpsimd SWDGE: even-ec first s-block slices, then the rest of
            #                the enc stream
            #   scalar ring: projs (needed by the first tanh), odd-ec first
            #                s-block slices, v, indicator
            w_sb = consts.tile([P, EC, H], BF16)
            for half in range(2):
                for ec in range(EC):
                    nc.sync.dma_start(
                        out=w_sb[:, ec, half * HH : (half + 1) * HH],
                        in_=w_t[half, ec],
                    )

            projs_sb = consts.tile([P, HC, BPC], F32)
            nc.scalar.dma_start(out=projs_sb[:], in_=projs_t[:])

            quads = []
            for _ in range(NSB):
                q = quad_psum.tile([P, SBLK], F32, tag="quad")
                # The collapse matmul reads all 128 partitions but the
                # v-rounds only ever write 4; zero once so stale PSUM
                # contents never reach the indicator dot.
                nc.vector.memset(q, 0.0)
                quads.append(q)

            # Deferred closures, emitted one main-MM group later so the PE
            # queue never reaches an instruction whose producer (ACT tanh or
            # DVE copy) hasn't had a full group (~1.7us) to finish.
            slots = {}

            def emit_slot(key):
                fns = slots.pop(key, None)
                if fns:
                    for fn in fns:
                        fn()

            def defer(key, fn):
                slots.setdefault(key, []).append(fn)

            v_sb = None
            ind_sb = None

            def dma_enc(b):
                encT = enc_pool.tile([P, EC, S], BF16)
                if b == 0:
                    # Only the first s-block; the bulk stream is emitted after
                    # a gpsimd throttle op so its HBM traffic cannot starve
                    # the weight descriptors during the prologue.
                    for ec in range(EC):
                        q = nc.gpsimd if ec % 2 == 0 else nc.scalar
                        q.dma_start(
                            out=encT[:, ec, 0:SBLK], in_=enc_t[0, ec, :, 0:SBLK]
                        )
                else:
                    for ec in range(EC):
                        nc.gpsimd.dma_start(out=encT[:, ec, :], in_=enc_t[b, ec])
                return encT

            for b in range(BPC):
                encT = dma_enc(b)
                if b == 0:
                    # v / indicator consts ride the scalar ring after the
                    # first s-block's enc slices (not needed until ~15us).
                    v_sb = consts.tile([P, HC, 1], BF16)
                    nc.scalar.dma_start(out=v_sb[:], in_=v_t[:])
                    ind_sb = consts.tile([P, P], BF16)
                    nc.scalar.dma_start(out=ind_sb[:], in_=ind_t[:])

                exp_row = row_pool.tile([1, S], F32, tag="exp_row")
                sums = row_pool.tile([1, NSB], F32, tag="sums")
                ths_b = []

                gi = 0  # main-MM group counter within this batch
                for sb in range(NSB):
                    ths = []
                    if b == 0 and sb == 0:
                        # ec-outer phase for the first two groups: banks from
                        # the (idle) collapse pool accumulate as the weight /
                        # enc descriptors land, so the PE starts on the first
                        # descriptor instead of the eighth.
                        ph = [
                            c_psum.tile([P, SBLK], F32, tag="cps", name=f"ph{k}")
                            for k in range(2)
                        ]
                        for ec in range(EC):
                            for hc in range(2):
                                nc.tensor.matmul(
                                    ph[hc],
                                    w_sb[:, ec, hc * P : (hc + 1) * P],
                                    encT[:, ec, 0:SBLK],
                                    start=(ec == 0),
                                    stop=(ec == EC - 1),
                                    skip_group_check=True,
                                )
                        for hc in range(2):
                            th = tanh_pool.tile([P, SBLK], BF16, tag="tanh")
                            nc.scalar.activation(
                                th, ph[hc], Tanh, bias=projs_sb[:, hc, 0:1]
                            )
                            ths.append(th)
                        # Throttle: the bulk enc stream may only start once
                        # the first tanh has run (~15us), by which point the
                        # 3MB hot set (weights + first s-block) has had the
                        # full HBM read bandwidth.
                        thr = row_pool.tile([1, 1], BF16, tag="thr")
                        nc.gpsimd.tensor_copy(thr, ths[0][0:1, 0:1])
                        for ec in range(EC):
                            nc.gpsimd.dma_start(
                                out=encT[:, ec, SBLK:S],
                                in_=enc_t[0, ec, :, SBLK:S],
                            )
                        hc_range = range(2, HC)
                    else:
                        hc_range = range(HC)
                    for hc in hc_range:
                        mm_ps = mm_psum.tile([P, SBLK], F32, tag="mmps")
                        for ec in range(EC):
                            nc.tensor.matmul(
                                mm_ps,
                                w_sb[:, ec, hc * P : (hc + 1) * P],
                                encT[:, ec, sb * SBLK : (sb + 1) * SBLK],
                                start=(ec == 0),
                                stop=(ec == EC - 1),
                            )
                        emit_slot((b, gi))
                        gi += 1
                        th = tanh_pool.tile([P, SBLK], BF16, tag="tanh")
                        nc.scalar.activation(
                            th, mm_ps, Tanh, bias=projs_sb[:, hc, b : b + 1]
                        )
                        ths.append(th)
                    ths_b.append(ths)

                def v_region(ths_b=ths_b, b=b):
                    for r in range(2):
                        for sb in range(NSB):
                            for j in range(4):
                                hc = 4 * r + j
                                nc.tensor.matmul(
                                    quads[sb][32 * j : 32 * j + 1, :],
                                    v_sb[:, hc, :],
                                    ths_b[sb][hc],
                                    start=(r == 0),
                                    stop=(r == 1),
                                    tile_position=(0, 32 * j),
                                    skip_group_check=True,
                                )

                def qcopy_sb(sb):
                    qc = qcopy_pool.tile([P, SBLK], BF16, tag="qcopy")
                    nc.vector.tensor_copy(qc, quads[sb])
                    return qc

                def collapse_sb(sb, qc, exp_row=exp_row, sums=sums):
                    sc = c_psum.tile([P, SBLK], F32, tag="cps")
                    nc.tensor.matmul(sc, ind_sb[:], qc, start=True, stop=True)
                    nc.scalar.activation(
                        exp_row[:, sb * SBLK : (sb + 1) * SBLK],
                        sc[0:1, :],
                        Exp,
                        accum_out=sums[:, sb : sb + 1],
                    )

                def batch_finale(b=b, exp_row=exp_row, sums=sums):
                    tot = row_pool.tile([1, 1], F32, tag="tot")
                    nc.vector.reduce_sum(tot, sums, axis=mybir.AxisListType.X)
                    rtot = row_pool.tile([1, 1], F32, tag="rtot")
                    nc.vector.reciprocal(rtot, tot)
                    out_row = row_pool.tile([1, S], F32, tag="out_row")
                    nc.vector.tensor_scalar_mul(out_row, exp_row, rtot)
                    nc.sync.dma_start(out=out[b : b + 1, :], in_=out_row[:])

                # schedule this batch's postlude into the next batch's
                # main-MM stream (group index g of batch b+1)
                nb = b + 1

                def sched(g, fn):
                    if nb < BPC:
                        defer((nb, g), fn)
                    else:
                        defer(("tail", 0), fn)

                qbox = {}
                sched(1, v_region)
                for i, sb in enumerate(range(NSB)):
                    sched(2 + i, (lambda sb=sb: qbox.__setitem__(sb, qcopy_sb(sb))))
                for i, sb in enumerate(range(NSB)):
                    sched(3 + i, (lambda sb=sb: collapse_sb(sb, qbox[sb])))
                sched(7, batch_finale)

            emit_slot(("tail", 0))

    if post:
        _dedup_ldweights(nc)
        _split_multiwaits(nc)
    return nc


def _prep_inputs(s, encoder_outputs, attn_w, v_w):
    s = np.asarray(s, dtype=np.float32)
    enc = np.asarray(encoder_outputs, dtype=np.float32)
    attn_w = np.asarray(attn_w, dtype=np.float32)
    v_w = np.asarray(v_w, dtype=np.float32)

    W_s = attn_w[:, :D]  # [H, D]
    W_e = attn_w[:, D:]  # [H, E]
    W_eT = np.ascontiguousarray(W_e.T)  # [E, H]
    # [2 halves, EC, P, 512]: descriptor (half, ec) is [128, 512] with 1KB
    # contiguous runs on both sides
    w_t = np.ascontiguousarray(
        W_eT.reshape(EC, P, 2, HH).transpose(2, 0, 1, 3)
    ).astype(NP_BF16)

    v_t = np.ascontiguousarray(v_w.reshape(HC, P).T).reshape(P, HC, 1).astype(NP_BF16)

    # indicator matrix: column 0 selects partitions {0,32,64,96}, all other
    # columns zero -> full-array collapse matmul writes the score into PSUM
    # row 0 and zeros elsewhere
    ind = np.zeros((P, P), dtype=NP_BF16)
    ind[[0, 32, 64, 96], 0] = 1.0

    projs = s @ W_s.T  # [B, H] fp32 on host (tiny)

    in_maps = []
    for c in range(N_CORES):
        lo, hi = c * BPC, (c + 1) * BPC
        enc_c = np.ascontiguousarray(enc[lo:hi].transpose(0, 2, 1)).astype(NP_BF16)
        enc_c = enc_c.reshape(BPC, EC, P, S)
        projs_c = np.ascontiguousarray(
            projs[lo:hi].T.reshape(HC, P, BPC).transpose(1, 0, 2)
        ).astype(np.float32)
        in_maps.append(
            {
                "enc_t": enc_c,
                "w_t": w_t,
                "projs_t": projs_c,
                "v_t": v_t,
                "ind_t": ind,
            }
        )
    return in_maps


def _run(s, encoder_outputs, attn_w, v_w, trace=False):
    if "nc" not in _cache:
        _cache["nc"] = _build_bass()
    nc = _cache["nc"]
    in_maps = _prep_inputs(s, encoder_outputs, attn_w, v_w)
    res = run_bass_kernel_spmd(nc, in_maps, list(range(N_CORES)), trace=trace)
    out = np.concatenate([res.results[c]["out"] for c in range(N_CORES)], axis=0)
    return out.astype(np.float32), res


def kernel(s, encoder_outputs, attn_w, v_w):
    out, _ = _run(s, encoder_outputs, attn_w, v_w, trace=False)
    return out


# revision 25
# speedup vs baseline: 1.1147x; 1.0024x over previous
"""Bahdanau-style attention scores kernel for Trainium2 (8 NeuronCores).

Reference computation (B=32, S=2048, ENC_H=512, DEC_H=1024):
    W_s = attn_w[:, :1024]; W_e = attn_w[:, 1024:]
    proj_s = s @ W_s.T                      # [B, 1024]
    proj_e = enc @ W_e.T                    # [B, S, 1024]
    scores = tanh(proj_s[:, None] + proj_e) @ v_w.T   # [B, S]
    out = softmax(scores, axis=1)

Strategy: data-parallel over batch (4 batches per core), no collectives.

Layout keeps the hidden dim h on PSUM partitions for the main matmul
    projT[h, s] = sum_e W_eT[e, h] * encT[e, s]
so the per-batch proj_s bias is a per-partition scalar fused into the ACT
tanh. proj_s is precomputed on the host (tiny), enc is pre-cast to bf16
on the host, the weight arrives as 16 half-row descriptors with 1KB
contiguous runs, and the first two PSUM accumulation groups run
ec-outer so the PE starts ~1us after the DMA rings open instead of
waiting for the whole weight.

The v-dot (scores = v . tanh): all 32 M=1 matmuls of one batch form one
contiguous col-group-tiled region (tile_position (0,32j)), so the four
XBUS streams pipeline at the full-array rate and the full<->col-group
drain penalty (~0.6us) is paid once per batch instead of per matmul
group. The four partial rows of each s-block (PSUM partitions
0/32/64/96) are collapsed by a zero-padded full-array matmul against a
4-hot indicator matrix (full-array so it slots into the main stream
with no transition cost). All cross-engine consumers are emitted one
main-MM group late (software pipelining) so the PE queue never waits on
ACT/DVE results.
"""

import numpy as np
import ml_dtypes

import concourse.bass as bass
import concourse.tile as tile
from concourse import mybir
from concourse.bass_utils import run_bass_kernel_spmd

N_CORES = 8
B, S = 32, 2048
E = 1024  # 2*ENC_H, contraction dim of the big matmul
H = 1024  # DEC_H, hidden dim of tanh
D = 1024  # DEC_H, contraction dim of proj_s
BPC = B // N_CORES  # batches per core
P = 128
EC, HC = E // P, H // P
SBLK = 512
NSB = S // SBLK
HH = H // 2  # weight DMA half-row

F32 = mybir.dt.float32
BF16 = mybir.dt.bfloat16
NP_BF16 = ml_dtypes.bfloat16

_cache = {}


def _split_multiwaits(nc):
    """Walrus in this toolchain rejects instructions carrying more than one
    semaphore wait ("Too many sync wait commands"). Engine queues dispatch in
    order, so moving the extra waits onto same-engine NoOps just before the
    instruction is semantically identical."""
    for fn in nc.m.functions:
        for blk in fn.blocks:
            out = []
            for inst in blk.instructions:
                si = inst.sync_info
                waits = list(si.on_wait) if si is not None and si.on_wait else []
                if len(waits) > 1:
                    for i, w in enumerate(waits[:-1]):
                        out.append(
                            mybir.InstNoOp(
                                name=f"{inst.name}-w{i}",
                                engine=inst.engine,
                                sync_info=mybir.SyncInfo(on_wait=[w], on_update=[]),
                                bass_nofuse=True,
                            )
                        )
                    si.on_wait = [waits[-1]]
                    inst.sync_info = si
                out.append(inst)
            try:
                blk.instructions = out
            except Exception:
                blk.set_instructions(out)


def _dedup_ldweights(nc):
    """Drop Ldweights that reload identical array state as the previous one
    (carrying their waits onto the next PE instruction)."""
    ndrop = 0
    for fn in nc.m.functions:
        for blk in fn.blocks:
            out = []
            loaded = None
            pending_waits = []
            for inst in blk.instructions:
                if getattr(inst, "engine", None) != mybir.EngineType.PE:
                    out.append(inst)
                    continue
                if pending_waits:
                    si = inst.sync_info or mybir.SyncInfo(on_wait=[], on_update=[])
                    si.on_wait = list(si.on_wait) + pending_waits
                    inst.sync_info = si
                    pending_waits = []
                if isinstance(inst, mybir.InstLdweights):
                    ap = inst.ins[0]
                    key = (
                        ap.memref,
                        ap.offset,
                        str(ap.ap),
                        str(ap.dtype),
                        str(getattr(inst, "tile_position", None)),
                    )
                    if key == loaded:
                        si = inst.sync_info
                        if si is not None and si.on_wait:
                            pending_waits = list(si.on_wait)
                        if si is not None and si.on_update:
                            out.append(inst)
                            continue
                        ndrop += 1
                        continue
                    loaded = key
                elif isinstance(inst, mybir.InstMatmult):
                    pass  # matmuls stream against loaded weights
                else:
                    loaded = None  # unknown PE instruction: be conservative
                out.append(inst)
            assert not pending_waits
            try:
                blk.instructions = out
            except Exception:
                blk.set_instructions(out)
    return ndrop


def _build_bass(post=True):
    nc = bass.Bass()
    enc_t = nc.dram_tensor("enc_t", [BPC, EC, P, S], BF16, kind="ExternalInput")
    # weight half-rows: [half, ec, p(e), 512(h)] so every descriptor writes
    # 1KB contiguous per partition
    w_t = nc.dram_tensor("w_t", [2, EC, P, HH], BF16, kind="ExternalInput")
    projs_t = nc.dram_tensor("projs_t", [P, HC, BPC], F32, kind="ExternalInput")
    v_t = nc.dram_tensor("v_t", [P, HC, 1], BF16, kind="ExternalInput")
    ind_t = nc.dram_tensor("ind_t", [P, P], BF16, kind="ExternalInput")
    out = nc.dram_tensor("out", [BPC, S], F32, kind="ExternalOutput")

    Tanh = mybir.ActivationFunctionType.Tanh
    Exp = mybir.ActivationFunctionType.Exp

    with tile.TileContext(nc) as tc:
        with (
            tc.tile_pool(name="consts", bufs=1) as consts,
            tc.tile_pool(name="enc", bufs=2) as enc_pool,
            tc.tile_pool(name="tanh", bufs=38) as tanh_pool,
            tc.tile_pool(name="qcopy", bufs=8) as qcopy_pool,
            tc.tile_pool(name="rows", bufs=2) as row_pool,
            tc.tile_pool(name="mmps", bufs=2, space="PSUM") as mm_psum,
            tc.tile_pool(name="quad", bufs=4, space="PSUM") as quad_psum,
            tc.tile_pool(name="cps", bufs=2, space="PSUM") as c_psum,
        ):
            # DMA queue plan (first use decides the critical path):
            #   sync ring:   16 weight half-row descriptors
            #   gpsimd SWDGE: even-ec first s-block slices, then the rest of
            #                the enc stream
            #   scalar ring: projs (needed by the first tanh), odd-ec first
            #                s-block slices, v, indicator
            w_sb = consts.tile([P, EC, H], BF16)
            for half in range(2):
                for ec in range(EC):
                    nc.sync.dma_start(
                        out=w_sb[:, ec, half * HH : (half + 1) * HH],
                        in_=w_t[half, ec],
                    )

            projs_sb = consts.tile([P, HC, BPC], F32)
            nc.scalar.dma_start(out=projs_sb[:], in_=projs_t[:])

            quads = []
            for _ in range(NSB):
                q = quad_psum.tile([P, SBLK], F32, tag="quad")
                # The collapse matmul reads all 128 partitions but the
                # v-rounds only ever write 4; zero once so stale PSUM
                # contents never reach the indicator dot.
                nc.vector.memset(q, 0.0)
                quads.append(q)

            # Deferred closures, emitted one main-MM group later so the PE
            # queue never reaches an instruction whose producer (ACT tanh or
            # DVE copy) hasn't had a full group (~1.7us) to finish.
            slots = {}

            def emit_slot(key):
                fns = slots.pop(key, None)
                if fns:
                    for fn in fns:
                        fn()

            def defer(key, fn):
                slots.setdefault(key, []).append(fn)

            v_sb = None
            ind_sb = None

            def dma_enc(b):
                encT = enc_pool.tile([P, EC, S], BF16)
                if b == 0:
                    # Only the first s-block; the bulk stream is emitted after
                    # a gpsimd throttle op so its HBM traffic cannot starve
                    # the weight descriptors during the prologue.
                    for ec in range(EC):
                        q = nc.gpsimd if ec % 2 == 0 else nc.scalar
                        q.dma_start(
                            out=encT[:, ec, 0:SBLK], in_=enc_t[0, ec, :, 0:SBLK]
                        )
                else:
                    for ec in range(EC):
                        nc.gpsimd.dma_start(out=encT[:, ec, :], in_=enc_t[b, ec])
                return encT

            for b in range(BPC):
                encT = dma_enc(b)
                if b == 0:
                    # v / indicator consts ride the scalar ring after the
                    # first s-block's enc slices (not needed until ~15us).
                    v_sb = consts.tile([P, HC, 1], BF16)
                    nc.scalar.dma_start(out=v_sb[:], in_=v_t[:])
                    ind_sb = consts.tile([P, P], BF16)
                    nc.scalar.dma_start(out=ind_sb[:], in_=ind_t[:])

                exp_row = row_pool.tile([1, S], F32, tag="exp_row")
                sums = row_pool.tile([1, NSB], F32, tag="sums")
                ths_b = []
                qbox = {}

                def v_region(sbs, ths_b=ths_b):
                    for r in range(2):
                        for sb in sbs:
                            for j in range(4):
                                hc = 4 * r + j
                                nc.tensor.matmul(
                                    quads[sb][32 * j : 32 * j + 1, :],
                                    v_sb[:, hc, :],
                                    ths_b[sb][hc],
                                    start=(r == 0),
                                    stop=(r == 1),
                                    tile_position=(0, 32 * j),
                                    skip_group_check=True,
                                )

                def qcopy_sb(sb, qbox=qbox):
                    qc = qcopy_pool.tile([P, SBLK], BF16, tag="qcopy")
                    nc.vector.tensor_copy(qc, quads[sb])
                    qbox[sb] = qc

                def collapse_sb(sb, qbox=qbox, exp_row=exp_row, sums=sums):
                    sc = c_psum.tile([P, SBLK], F32, tag="cps")
                    nc.tensor.matmul(sc, ind_sb[:], qbox[sb], start=True, stop=True)
                    nc.scalar.activation(
                        exp_row[:, sb * SBLK : (sb + 1) * SBLK],
                        sc[0:1, :],
                        Exp,
                        accum_out=sums[:, sb : sb + 1],
                    )

                def batch_finale(b=b, exp_row=exp_row, sums=sums):
                    tot = row_pool.tile([1, 1], F32, tag="tot")
                    nc.vector.reduce_sum(tot, sums, axis=mybir.AxisListType.X)
                    rtot = row_pool.tile([1, 1], F32, tag="rtot")
                    nc.vector.reciprocal(rtot, tot)
                    out_row = row_pool.tile([1, S], F32, tag="out_row")
                    nc.vector.tensor_scalar_mul(out_row, exp_row, rtot)
                    nc.sync.dma_start(out=out[b : b + 1, :], in_=out_row[:])

                if b == BPC - 1:
                    # Last batch: s-blocks 0-2 collapse inside this batch's
                    # own main stream (their tanhs are long done); only
                    # s-block 3's short chain trails the final main matmul.
                    defer((b, 26), lambda: v_region(range(3)))
                    defer((b, 27), lambda: qcopy_sb(0))
                    defer((b, 28), lambda: qcopy_sb(1))
                    defer((b, 28), lambda: collapse_sb(0))
                    defer((b, 29), lambda: qcopy_sb(2))
                    defer((b, 29), lambda: collapse_sb(1))
                    defer((b, 30), lambda: collapse_sb(2))
                    defer(("tail", 0), lambda: v_region([3]))
                    defer(("tail", 0), lambda: qcopy_sb(3))
                    defer(("tail", 0), lambda: collapse_sb(3))
                    defer(("tail", 0), batch_finale)

                gi = 0  # main-MM group counter within this batch
                for sb in range(NSB):
                    ths = []
                    if b == 0 and sb == 0:
                        # ec-outer phase for the first two groups: banks from
                        # the (idle) collapse pool accumulate as the weight /
                        # enc descriptors land, so the PE starts on the first
                        # descriptor instead of the eighth.
                        ph = [
                            c_psum.tile([P, SBLK], F32, tag="cps", name=f"ph{k}")
                            for k in range(2)
                        ]
                        for ec in range(EC):
                            for hc in range(2):
                                nc.tensor.matmul(
                                    ph[hc],
                                    w_sb[:, ec, hc * P : (hc + 1) * P],
                                    encT[:, ec, 0:SBLK],
                                    start=(ec == 0),
                                    stop=(ec == EC - 1),
                                    skip_group_check=True,
                                )
                        for hc in range(2):
                            th = tanh_pool.tile([P, SBLK], BF16, tag="tanh")
                            nc.scalar.activation(
                                th, ph[hc], Tanh, bias=projs_sb[:, hc, 0:1]
                            )
                            ths.append(th)
                        # Throttle: the bulk enc stream may only start once
                        # the first tanh has run (~15us), by which point the
                        # 3MB hot set (weights + first s-block) has had the
                        # full HBM read bandwidth.
                        thr = row_pool.tile([1, 1], BF16, tag="thr")
                        nc.gpsimd.tensor_copy(thr, ths[0][0:1, 0:1])
                        for ec in range(EC):
                            nc.gpsimd.dma_start(
                                out=encT[:, ec, SBLK:S],
                                in_=enc_t[0, ec, :, SBLK:S],
                            )
                        hc_range = range(2, HC)
                    else:
                        hc_range = range(HC)
                    for hc in hc_range:
                        mm_ps = mm_psum.tile([P, SBLK], F32, tag="mmps")
                        for ec in range(EC):
                            nc.tensor.matmul(
                                mm_ps,
                                w_sb[:, ec, hc * P : (hc + 1) * P],
                                encT[:, ec, sb * SBLK : (sb + 1) * SBLK],
                                start=(ec == 0),
                                stop=(ec == EC - 1),
                            )
                        emit_slot((b, gi))
                        gi += 1
                        th = tanh_pool.tile([P, SBLK], BF16, tag="tanh")
                        nc.scalar.activation(
                            th, mm_ps, Tanh, bias=projs_sb[:, hc, b : b + 1]
                        )
                        ths.append(th)
                    ths_b.append(ths)

                if b < BPC - 1:
                    # schedule this batch's postlude into the next batch's
                    # main-MM stream (group index g of batch b+1)
                    nb = b + 1
                    defer((nb, 1), lambda vr=v_region: vr(range(NSB)))
                    for i in range(NSB):
                        defer((nb, 2 + i), (lambda sb=i, f=qcopy_sb: f(sb)))
                    for i in range(NSB):
                        defer((nb, 3 + i), (lambda sb=i, f=collapse_sb: f(sb)))
                    defer((nb, 7), batch_finale)

            emit_slot(("tail", 0))

    if post:
        _dedup_ldweights(nc)
        _split_multiwaits(nc)
    return nc


def _prep_inputs(s, encoder_outputs, attn_w, v_w):
    s = np.asarray(s, dtype=np.float32)
    enc = np.asarray(encoder_outputs, dtype=np.float32)
    attn_w = np.asarray(attn_w, dtype=np.float32)
    v_w = np.asarray(v_w, dtype=np.float32)

    W_s = attn_w[:, :D]  # [H, D]
    W_e = attn_w[:, D:]  # [H, E]
    W_eT = np.ascontiguousarray(W_e.T)  # [E, H]
    # [2 halves, EC, P, 512]: descriptor (half, ec) is [128, 512] with 1KB
    # contiguous runs on both sides
    w_t = np.ascontiguousarray(
        W_eT.reshape(EC, P, 2, HH).transpose(2, 0, 1, 3)
    ).astype(NP_BF16)

    v_t = np.ascontiguousarray(v_w.reshape(HC, P).T).reshape(P, HC, 1).astype(NP_BF16)

    # indicator matrix: column 0 selects partitions {0,32,64,96}, all other
    # columns zero -> full-array collapse matmul writes the score into PSUM
    # row 0 and zeros elsewhere
    ind = np.zeros((P, P), dtype=NP_BF16)
    ind[[0, 32, 64, 96], 0] = 1.0

    projs = s @ W_s.T  # [B, H] fp32 on host (tiny)

    in_maps = []
    for c in range(N_CORES):
        lo, hi = c * BPC, (c + 1) * BPC
        enc_c = np.ascontiguousarray(enc[lo:hi].transpose(0, 2, 1)).astype(NP_BF16)
        enc_c = enc_c.reshape(BPC, EC, P, S)
        projs_c = np.ascontiguousarray(
            projs[lo:hi].T.reshape(HC, P, BPC).transpose(1, 0, 2)
        ).astype(np.float32)
        in_maps.append(
            {
                "enc_t": enc_c,
                "w_t": w_t,
                "projs_t": projs_c,
                "v_t": v_t,
                "ind_t": ind,
            }
        )
    return in_maps


def _run(s, encoder_outputs, attn_w, v_w, trace=False):
    if "nc" not in _cache:
        _cache["nc"] = _build_bass()
    nc = _cache["nc"]
    in_maps = _prep_inputs(s, encoder_outputs, attn_w, v_w)
    res = run_bass_kernel_spmd(nc, in_maps, list(range(N_CORES)), trace=trace)
    out = np.concatenate([res.results[c]["out"] for c in range(N_CORES)], axis=0)
    return out.astype(np.float32), res


def kernel(s, encoder_outputs, attn_w, v_w):
    out, _ = _run(s, encoder_outputs, attn_w, v_w, trace=False)
    return out
